# revision 22
# baseline (speedup 1.0000x reference)
"""Chamfer-distance kernel for 8 Trainium2 NeuronCores (Bass/Tile).

Problem: gts [8, 8192, 3] f32, preds [8, 8192, 3] f32 ->
         scalar chamfer distance (pytorch3d convention: squared L2,
         mean over points, mean over batch, sum of both directions).

Sharding: one batch element per NeuronCore (B == n_cores == 8).

Algorithm (pruned exact nearest-neighbour):
  Host:
  - kd-sort each query cloud into 256 spatially tight sub-tiles of 32
    points (median splits -> perfectly balanced leaves).
  - For each sub-tile, select the KP candidate points of the other
    cloud with the smallest point-to-subtile-bbox distance (a true
    lower bound of the distance to any query in the sub-tile), and
    record the (KP+1)-th smallest bound as a certificate threshold.
  - Pack queries/candidates into K=24-row bf16 triple-split matmul
    operands such that sum_k xq[k,n]*yc[k,m] = d2(x_n, y_m) at
    fp32-class accuracy.
  Device (per direction, per group of 16 sub-tiles = 512 queries):
  - 16 column-tiled matmuls [24,32]x[24,KP] (tile_position=(0,32c))
    fill a PSUM group [128, 4, 512pad] f32: bank j, partition block c
    holds sub-tile 4j+c's d2 rows against its own candidates.
  - Route A (most groups): ScalarE copies the group to fp16 SBUF;
    VectorE halving-min-folds KP->KP/8 at 2x rate, then one segmented
    tensor_reduce -> per-query min into a [128, 64] output column block.
  - Route B (some groups): VectorE reduces the PSUM group directly (1x)
    with a segmented tensor_reduce, freeing ScalarE.  The mix keeps
    both engines near-equally busy.
  Host epilogue:
  - For the few points whose device min exceeds the certificate
    threshold (the candidate set may not contain the true NN), compute
    the exact row against the full cloud.  Result is exact up to
    rounding; clamp at 0 and average.
"""

import os
import sys

sys.path.insert(0, "/opt/trn_rl_repo")

import numpy as np
import ml_dtypes

import concourse.bacc as bacc
import concourse.mybir as mybir
import concourse.tile as tile
from concourse.bass_utils import run_bass_kernel_spmd

BF16 = ml_dtypes.bfloat16

B = 8
NPTS = 8192
GRAIN = int(os.environ.get("CHAMFER_GRAIN", "64"))  # queries per sub-tile
NSUB = NPTS // GRAIN       # sub-tiles per direction
NB = 128 // GRAIN          # sub-tiles (col-tiles) per PSUM bank
LEAF_POW = NSUB.bit_length() - 1
KP = int(os.environ.get("CHAMFER_KP", "192"))  # candidates per sub-tile
KROWS = 24                 # packed contraction rows
SPG = 4 * NB               # sub-tiles per PSUM group (512 queries)
NGROUPS = NSUB // SPG      # 16 groups per direction
# every n-th group uses the ScalarE-copy route (0 = never: pure DVE route B)
ROUTE_A_EVERY = int(os.environ.get("CHAMFER_RAE", "0"))
ROT2 = int(os.environ.get("CHAMFER_ROT2", "1"))  # 2-way PE row rotation
HROWS = 56 if ROT2 else KROWS


# ----------------------------------------------------------------- host prep

def _kd_order(pts, leaf_pow):
    """Recursive median split -> permutation; 2**leaf_pow equal leaves."""
    out = []

    def rec(ids, depth):
        if depth == 0:
            out.append(ids)
            return
        p = pts[ids]
        dim = int(np.argmax(p.max(0) - p.min(0)))
        half = len(ids) // 2
        part = np.argpartition(p[:, dim], half)
        rec(ids[part[:half]], depth - 1)
        rec(ids[part[half:]], depth - 1)

    rec(np.arange(len(pts)), leaf_pow)
    return np.concatenate(out)


def _split3(v):
    """float64 vector -> three bf16-representable float64 components."""
    a0 = v.astype(BF16).astype(np.float64)
    a1 = (v - a0).astype(BF16).astype(np.float64)
    a2 = (v - a0 - a1).astype(BF16).astype(np.float64)
    return a0, a1, a2

_TERMS = [(0, 0), (0, 1), (1, 0), (1, 1), (0, 2), (2, 0)]


def _pack_queries(x):
    """x [N,3] f32 -> [24, N] bf16 query-side operand."""
    xd = x.astype(np.float64)
    ax = [_split3(xd[:, d]) for d in range(3)]
    x2 = _split3((xd * xd).sum(1))
    n = x.shape[0]
    xp = np.zeros((KROWS, n), dtype=BF16)
    r = 0
    for (i, _) in _TERMS:
        for d in range(3):
            xp[r] = (-2.0 * ax[d][i]).astype(BF16)
            r += 1
    for k in range(3):
        xp[r] = x2[k].astype(BF16)
        r += 1
    xp[r:r + 3] = np.ones((3, n), dtype=BF16)
    return xp


def _pack_cands(y):
    """y [M,3] f32 -> [24, M] bf16 candidate-side operand."""
    yd = y.astype(np.float64)
    by = [_split3(yd[:, d]) for d in range(3)]
    y2 = _split3((yd * yd).sum(1))
    m = y.shape[0]
    yp = np.zeros((KROWS, m), dtype=BF16)
    r = 0
    for (_, j) in _TERMS:
        for d in range(3):
            yp[r] = by[d][j].astype(BF16)
            r += 1
    yp[r:r + 3] = np.ones((3, m), dtype=BF16)
    r += 3
    for k in range(3):
        yp[r] = y2[k].astype(BF16)
        r += 1
    return yp


def _rot_pack(a):
    """[24, N] -> [HROWS, N] with rows duplicated at offset 32 (ROT2)."""
    if not ROT2:
        return np.ascontiguousarray(a)
    out = np.zeros((HROWS, a.shape[1]), dtype=a.dtype)
    out[:KROWS] = a
    out[32:32 + KROWS] = a
    return out


def _prep_direction(q, y):
    """Queries q [8192,3] vs candidate cloud y [8192,3].

    Returns (xq [H,8192] bf16, yc [H, NSUB*KP] bf16, xs [8192,3] sorted
    queries, kth [NSUB] certificate thresholds)."""
    ox = _kd_order(q, LEAF_POW)
    xs = q[ox]
    xt = xs.reshape(NSUB, GRAIN, 3)
    tlo = xt.min(1)
    thi = xt.max(1)
    g = np.maximum(0.0, np.maximum(tlo[:, None] - y[None], y[None] - thi[:, None]))
    lbp = (g * g).sum(-1)                                  # [NSUB, M]
    part = np.argpartition(lbp, KP, axis=1)
    sel = part[:, :KP]                                     # [NSUB, KP]
    kth = np.take_along_axis(lbp, part[:, KP:KP + 1], axis=1)[:, 0]  # [NSUB]
    xq = _pack_queries(xs)
    ypk = _pack_cands(y)
    yc = ypk[:, sel.reshape(-1)]                           # [24, NSUB*KP]
    return _rot_pack(xq), _rot_pack(yc), xs, kth


def prepare_in_maps(gts, preds):
    """Returns (in_maps for run_bass_kernel_spmd, meta for the epilogue)."""
    in_maps, metas = [], []
    for b in range(B):
        xq0, yc0, xs0, kth0 = _prep_direction(gts[b], preds[b])
        xq1, yc1, xs1, kth1 = _prep_direction(preds[b], gts[b])
        in_maps.append({"xq0": xq0, "yc0": yc0, "xq1": xq1, "yc1": yc1})
        metas.append(((xs0, kth0, preds[b]), (xs1, kth1, gts[b])))
    return in_maps, metas


# ------------------------------------------------------------- device kernel

def _build_nc(reps=1):
    f32 = mybir.dt.float32
    f16 = mybir.dt.float16
    bf16 = mybir.dt.bfloat16
    MIN = mybir.AluOpType.min
    X = mybir.AxisListType.X
    nc = bacc.Bacc()

    xq_d = [nc.declare_dram_parameter(f"xq{d}", [HROWS, NPTS], bf16,
                                      isOutput=False) for d in range(2)]
    yc_d = [nc.declare_dram_parameter(f"yc{d}", [HROWS, NSUB * KP], bf16,
                                      isOutput=False) for d in range(2)]
    v_d = [nc.declare_dram_parameter(f"v{d}", [128, 4 * NGROUPS], f32,
                                     isOutput=True) for d in range(2)]

    with tile.TileContext(nc) as tc:
        with (
            tc.tile_pool(name="xq", bufs=1) as xqp,
            tc.tile_pool(name="yc", bufs=3) as ycp,
            tc.tile_pool(name="sb", bufs=3) as sbp,
            tc.tile_pool(name="vout", bufs=1) as vp,
            tc.tile_pool(name="ps", bufs=2, space="PSUM") as psp,
        ):
            xqs = [xqp.tile([HROWS, NPTS], bf16, name=f"xqs{d}")
                   for d in range(2)]
            for d in range(2):
                nc.sync.dma_start(xqs[d][:], xq_d[d][:])
            vts = [vp.tile([128, 4 * NGROUPS], f32, name=f"vts{d}")
                   for d in range(2)]

            def body():
                for d in range(2):
                    for g in range(NGROUPS):
                        yc = ycp.tile([HROWS, SPG * KP], bf16, name="yc")
                        src = yc_d[d][:, g * SPG * KP:(g + 1) * SPG * KP]
                        (nc.sync if g % 2 == 0 else nc.gpsimd).dma_start(
                            yc[:], src)
                        # PSUM rows bank-padded to 512 f32
                        ps = psp.tile([128, 4, 512], f32, name="ps")
                        for j in range(4):
                            for c in range(NB):
                                s = j * NB + c     # sub-tile within group
                                ro = 32 * (s % 2) if ROT2 else 0
                                nc.tensor.matmul(
                                    ps[GRAIN * c:GRAIN * (c + 1), j, :KP],
                                    xqs[d][ro:ro + KROWS,
                                           (g * SPG + s) * GRAIN:
                                           (g * SPG + s + 1) * GRAIN],
                                    yc[ro:ro + KROWS, s * KP:(s + 1) * KP],
                                    start=True, stop=True,
                                    tile_position=(ro, GRAIN * c))
                        t0 = g * 4
                        if ROUTE_A_EVERY == 0 or g % ROUTE_A_EVERY:
                            # direct segmented min-reduce from PSUM (DVE 1x)
                            nc.vector.tensor_reduce(
                                vts[d][:, t0:t0 + 4], ps[:, :, :KP],
                                axis=X, op=MIN)
                        else:
                            # ScalarE evacuates to fp16; DVE folds at 2x
                            sb = sbp.tile([128, 4, KP], f16, name="sb")
                            nc.scalar.copy(sb[:], ps[:, :, :KP])
                            h = KP // 2
                            nc.vector.tensor_tensor(
                                sb[:, :, :h], sb[:, :, :h], sb[:, :, h:],
                                op=MIN)
                            nc.vector.tensor_tensor(
                                sb[:, :, :h // 2], sb[:, :, :h // 2],
                                sb[:, :, h // 2:h], op=MIN)
                            nc.vector.tensor_tensor(
                                sb[:, :, :h // 4], sb[:, :, :h // 4],
                                sb[:, :, h // 4:h // 2], op=MIN)
                            nc.vector.tensor_reduce(
                                vts[d][:, t0:t0 + 4],
                                sb[:, :, :h // 4], axis=X, op=MIN)

            if reps == 1:
                body()
            else:
                with tc.For_i(0, reps, 1):
                    body()

            for d in range(2):
                nc.sync.dma_start(v_d[d][:], vts[d][:])
    nc.compile()
    return nc


_CACHED_NC = None


def _get_nc():
    global _CACHED_NC
    if _CACHED_NC is None:
        _CACHED_NC = _build_nc(reps=1)
    return _CACHED_NC


# ------------------------------------------------------------------ epilogue

def _direction_mean(v, xs, kth, y):
    """v [128, 64] device mins; v.T.reshape(-1) is sorted-query order.

    Fix up points whose min is not certified optimal, clamp, return mean."""
    mins = v.T.reshape(-1).astype(np.float64)              # sorted order
    thresh = np.repeat(kth, GRAIN)
    suspect = mins * 1.01 + 1e-5 > thresh
    idx = np.nonzero(suspect)[0]
    yf = y.astype(np.float32)
    for i in range(0, len(idx), 1024):
        ii = idx[i:i + 1024]
        d2 = ((xs[ii][:, None, :].astype(np.float32)
               - yf[None, :, :]) ** 2).sum(-1)
        mins[ii] = np.minimum(mins[ii], d2.min(1).astype(np.float64))
    return np.maximum(mins, 0.0).mean()


def kernel(gts, preds):
    gts = np.asarray(gts, dtype=np.float32)
    preds = np.asarray(preds, dtype=np.float32)
    assert gts.shape == (B, NPTS, 3) and preds.shape == (B, NPTS, 3), (
        gts.shape, preds.shape)

    nc = _get_nc()
    in_maps, metas = prepare_in_maps(gts, preds)
    res = run_bass_kernel_spmd(nc, in_maps, list(range(B)))

    total = 0.0
    for b in range(B):
        for d in range(2):
            xs, kth, y = metas[b][d]
            v = res.results[b][f"v{d}"]
            total += _direction_mean(v, xs, kth, y)
    return np.float32(total / B)
